# revision 77
# baseline (speedup 1.0000x reference)
"""Causal self-attention (B=4, S=2048, D=1024, H=16, hd=64) on 8 TRN2 NeuronCores.

Sharding: batch 4-way x head-group 2-way. Core c = 2*b + g handles batch b and
heads [8g, 8g+8). Each core computes the QKV projection for its heads, causal
flash-style attention, and a partial output projection; the host sums the two
head-group partials per batch.

v3 schedule -- a software-pipelined per-chunk stream:
  - Attention runs over a flattened (head-pair, kv-chunk) stream with a
    one-chunk lookahead: QK(c+1) is issued on the PE queue BEFORE AV(c), so
    the next chunk's scores stream while this chunk's exp runs on ACT,
    including across head-pair boundaries. Score tiles are per-chunk
    [P, head, QSB] (slab = head, adjacent psum banks), freed by their own
    single two-head exp -- 2 score bufs suffice for the lookahead, and exp
    never strictly alternates with QK.
  - A global deque of ~0.85us filler granules (projection half-contraction
    pieces, deferred out-projection halves) keeps the PE saturated and its
    p-state at max. Fillers are appended in each superblock's demand order
    and tagged by the data they publish (qkT rows, v chunks); consumers
    demand-pop via ensure() so correctness never depends on pacing, while a
    capped 4-iteration lookahead pre-pop plus a 3-of-4 background drain
    keeps the deque smooth.
  - Only the first head-pair's stripe-0 pieces run inline at startup; the
    rest demand-pop behind the DMA stream.
  - The sb3 out-projection's first 3 contraction chunks pre-start inside the
    final normalize (they only need head-pairs 0-2), covering the
    reciprocal chain so the tail runs at full clock.
Attention per (head-pair, chunk): S^T = K.Q^T lands P^T via exp() in the
layout the P^T.V matmul wants; a ones-column in V yields softmax denominators
(row 64). Head pairs sit on disjoint PE row halves so their QK matmuls stream
concurrently (PE row-group tiling). No running-max: scores are bounded, exp
stays finite in fp32.
"""

import sys

for _p in ("/opt/trn_rl_repo",):
    if _p not in sys.path:
        sys.path.insert(0, _p)

from contextlib import ExitStack

import numpy as np

import concourse.bass as bass
import concourse.mybir as mybir
import concourse.tile as tile
from concourse import bacc
from concourse.bass_utils import run_bass_kernel_spmd

F32 = mybir.dt.float32
BF16 = mybir.dt.bfloat16
P = 128
B, S, D = 4, 2048, 1024
HD = 64          # head dim
NH = 8           # heads per core
KO = D // P      # 8 contraction chunks for the projections
QSB = 512        # q superblock (matmul free dim)
N_SB = S // QSB  # 4
N_SC = S // P    # 16 kv chunks
PSTRIPE = 512    # s-stripe for the projection phase
SCALE = 0.125    # 1/sqrt(64)


def _attention_kernel(tc, out, xT, w_qk, w_v, w_out):
    nc = tc.nc
    with ExitStack() as ctx:
        const_pool = ctx.enter_context(tc.tile_pool(name="const", bufs=1))
        qkT_pool = ctx.enter_context(tc.tile_pool(name="qkT", bufs=1))
        v_pool = ctx.enter_context(tc.tile_pool(name="vsb", bufs=1))
        wqk_pool = ctx.enter_context(tc.tile_pool(name="wqk", bufs=1))
        wv_pool = ctx.enter_context(tc.tile_pool(name="wv", bufs=1))
        wout_pool = ctx.enter_context(tc.tile_pool(name="wout", bufs=1))
        xt_pool = ctx.enter_context(tc.tile_pool(name="xt", bufs=3))
        pt_pool = ctx.enter_context(tc.tile_pool(name="pt", bufs=6))
        y_pool = ctx.enter_context(tc.tile_pool(name="ysb", bufs=2))
        r_pool = ctx.enter_context(tc.tile_pool(name="recip", bufs=6))
        o_pool = ctx.enter_context(tc.tile_pool(name="osb", bufs=3))
        # PSUM: scores 2x2 banks + work(proj/out) 2x1 + y accum 2x1 = 8 banks
        ps_sc = ctx.enter_context(tc.tile_pool(name="ps_sc", bufs=2, space="PSUM"))
        ps_wk = ctx.enter_context(tc.tile_pool(name="ps_wk", bufs=2, space="PSUM"))
        ps_y = ctx.enter_context(tc.tile_pool(name="ps_y", bufs=2, space="PSUM"))

        # 128x128 triangle for the diagonal block (transposed layout):
        # tri[i, j] = 1 if j >= i else 0
        tri = const_pool.tile([P, P], BF16, tag="tri")
        nc.gpsimd.memset(tri[:], 1.0)
        ones64 = const_pool.tile([1, HD], BF16, tag="ones64")
        nc.gpsimd.memset(ones64[:], 1.0)
        nc.gpsimd.affine_select(
            out=tri[:],
            in_=tri[:],
            compare_op=mybir.AluOpType.is_ge,
            fill=0.0,
            base=0,
            channel_multiplier=-1,
            pattern=[[1, P]],
        )

        # p-state warmup: junk matmuls with no DMA dependencies keep the
        # PE busy (and its clock ramping) while the first inputs land.
        warm_w = const_pool.tile([P, P], BF16, tag="warmw")
        nc.gpsimd.memset(warm_w[:], 1.0)
        warm_x = const_pool.tile([P, 512], BF16, tag="warmx")
        nc.gpsimd.memset(warm_x[:], 1.0)
        warm_ps = ps_y.tile([P, 512], F32, tag="ps_y", name="warmps")
        # q^T/k^T store: row-chunk rc<4 holds q rows, rc>=4 holds k rows.
        # Head h lives at partitions 64*(h%2)..+64 of row-chunk h//2 (+4 for k).
        qkT = qkT_pool.tile([P, 8, S], BF16)
        for _w in range(10):
            nc.tensor.matmul(warm_ps[:], lhsT=warm_w[:], rhs=warm_x[:],
                             start=True, stop=True)
        # V store: [s-partition, kv-chunk, head, hd+1]; last col is ones for the
        # softmax denominator.
        v_sb = v_pool.tile([P, N_SC, NH, HD + 1], BF16)
        nc.gpsimd.memset(v_sb[:, :, :, HD], 1.0)

        # Startup loads: wqk rides full-width rows (2KB/partition DMA lines
        # run ~1.7x faster than the 1KB halves), so q+k weights land in
        # ~5.7us instead of 9.4.
        wqk_sb = wqk_pool.tile([P, KO, 2 * 512], BF16)
        xts = [None] * 4
        xts[0] = xt_pool.tile([P, KO, PSTRIPE], BF16, tag="xt", name="xt0")
        for ko in range(KO):
            nc.sync.dma_start(
                wqk_sb[:, ko, :],
                w_qk[ko * P:(ko + 1) * P, :],
            )
        wv_sb = wv_pool.tile([P, KO, 512], BF16)
        for ko in range(KO):
            nc.sync.dma_start(xts[0][:, ko, :],
                              xT[ko * P:(ko + 1) * P, 0:PSTRIPE])
        for ko in range(KO):
            nc.sync.dma_start(wv_sb[:, ko, :], w_v[ko * P:(ko + 1) * P, :])
        wout_sb = wout_pool.tile([P, 4, D], BF16)
        for co in range(4):
            nc.sync.dma_start(wout_sb[:, co, :], w_out[co * P:(co + 1) * P, :])

        def load_stripe(st):
            xt = xt_pool.tile([P, KO, PSTRIPE], BF16, tag="xt", name=f"xt{st}")
            for ko in range(KO):
                nc.sync.dma_start(
                    xt[:, ko, :],
                    xT[ko * P:(ko + 1) * P, st * PSTRIPE:(st + 1) * PSTRIPE],
                )
            xts[st] = xt

        # ---- filler units: closures of ~0.9-1.9us of PE work ----

        def qk_rc(st, rc, alt=False):
            """q^T/k^T rows for row-chunk rc of stripe st: two ~0.85us
            closures (half-contraction granules) so fillers never overshoot
            the per-iteration PE slack."""
            cell = {}

            def half(koh):
                if koh == 0:
                    pool, tg = (ps_y, "ps_y") if alt else (ps_wk, "ps_wk")
                    cell[0] = pool.tile(
                        [P, PSTRIPE], F32, tag=tg, name=f"pqk{st}_{rc}"
                    )
                ps = cell[0]
                for ko in range(4 * koh, 4 * koh + 4):
                    nc.tensor.matmul(
                        ps[:],
                        lhsT=wqk_sb[:, ko, rc * P:(rc + 1) * P],
                        rhs=xts[st][:, ko, :],
                        start=(ko == 0),
                        stop=(ko == KO - 1),
                    )
                if koh == 1:
                    nc.vector.tensor_copy(
                        qkT[:, rc, st * PSTRIPE:(st + 1) * PSTRIPE], ps[:]
                    )

            return [(None, lambda: half(0)),
                    (("qk", st, rc), lambda: half(1))]

        def v_sub(st, sub, alt=False):
            """V rows for s-chunk 4*st+sub: two ~0.85us closures."""
            cell = {}

            def half(koh):
                if koh == 0:
                    pool, tg = (ps_y, "ps_y") if alt else (ps_wk, "ps_wk")
                    cell[0] = pool.tile(
                        [P, NH * HD], F32, tag=tg, name=f"pv{st}_{sub}"
                    )
                ps = cell[0]
                for ko in range(4 * koh, 4 * koh + 4):
                    nc.tensor.matmul(
                        ps[:],
                        lhsT=xts[st][:, ko, sub * P:(sub + 1) * P],
                        rhs=wv_sb[:, ko, :],
                        start=(ko == 0),
                        stop=(ko == KO - 1),
                    )
                if koh == 1:
                    sc = st * (PSTRIPE // P) + sub
                    nc.vector.tensor_copy(
                        v_sb[:, sc, :, 0:HD],
                        ps.rearrange("p (h e) -> p h e", h=NH),
                    )

            return [(None, lambda: half(0)),
                    (("v", st * 4 + sub), lambda: half(1))]

        def stripe_fillers(st):
            """All of stripe st's proj pieces in the demand order of
            superblock st's attention: q/k rows for hp0 first, then the v
            chunks (needed from iteration 4*hp0+...), then later head-pairs'
            rows interleaved q,k."""
            out = []
            out += qk_rc(st, 0) + qk_rc(st, 4)
            for sub in range(4):
                out += v_sub(st, sub)
            for hp in range(1, 4):
                out += qk_rc(st, hp) + qk_rc(st, 4 + hp)
            return out

        ySbs = [None] * N_SB

        def out_unit(sb, sub):
            """Output projection for s-rows sb*512+sub*128..+128. Two closures."""
            cell = {}

            def half(nt):
                ps = ps_wk.tile([P, 512], F32, tag="ps_wk", name=f"ops{sb}_{sub}_{nt}")
                ySb = ySbs[sb]
                for cc in range(4):
                    nc.tensor.matmul(
                        ps[:],
                        lhsT=ySb[:, cc, sub * P:(sub + 1) * P],
                        rhs=wout_sb[:, cc, nt * 512:(nt + 1) * 512],
                        start=(cc == 0),
                        stop=(cc == 3),
                    )
                if nt == 0:
                    cell["o_t"] = o_pool.tile([P, 2, 512], F32, tag="osb", name=f"ot{sb}_{sub}")
                o_t = cell["o_t"]
                nc.vector.tensor_copy(o_t[:, nt, :], ps[:])
                if nt == 1:
                    row = (sb * (QSB // P) + sub) * P
                    nc.sync.dma_start(
                        out[row:row + P, :], o_t.rearrange("p a b -> p (a b)")
                    )

            return [(None, lambda: half(0)), (None, lambda: half(1))]

        # global filler deque of (tag, closure); tags mark closures that
        # publish data (qkT rows, v chunks) so consumers can demand-pop
        # them before issuing reads -- correctness no longer depends on
        # pop pacing.
        fillers = []
        done_tags = set()

        def _run_filler():
            tag, fn = fillers.pop(0)
            fn()
            if tag is not None:
                done_tags.add(tag)
            return tag

        def pop_fillers(n):
            for _ in range(n):
                if fillers:
                    _run_filler()

        def flush_fillers():
            while fillers:
                _run_filler()

        def ensure(tag):
            if tag in done_tags:
                return
            while fillers:
                if _run_filler() == tag:
                    return

        pending = []   # deferred normalize multiplies (previous head-pair)

        def attn_sb(sb, tail_reserve=0, pre_norm_hook=None, bg_period=2):
            ySb = y_pool.tile([P, 4, QSB], BF16, tag="ysb", name=f"ysb{sb}")
            ySbs[sb] = ySb
            nch = 4 * (sb + 1)
            ys = {}

            # One-chunk software pipeline over a flattened (head-pair, chunk)
            # stream: QK(next) is issued on the PE queue BEFORE AV(cur), so
            # the next chunk's scores stream while this chunk's exp runs on
            # ACT -- including across head-pair boundaries. A chunk's score
            # tile [P, head, QSB] frees as soon as its own (single, two-head)
            # exp retires, so 2 score bufs suffice for the lookahead.
            def issue_chunk(hp, c):
                heads = (2 * hp, 2 * hp + 1)
                rc_k = 4 + hp
                qo = P * max(0, c - 4 * sb)
                # demand-pop the fillers that publish this chunk's inputs
                ensure(("qk", sb, hp))        # q rows, stripe sb
                ensure(("qk", c // 4, rc_k))  # k rows, chunk stripe
                ensure(("v", c))              # v rows (read 1 iter later)
                s2c = ps_sc.tile([P, 2, QSB], F32, tag="ps_sc",
                                 name=f"s2c{c % 2}")
                for j, h in enumerate(heads):
                    bp = (h % 2) * HD
                    nc.tensor.matmul(
                        s2c[:, j, qo:],
                        lhsT=qkT[bp:bp + HD, rc_k, c * P:(c + 1) * P],
                        rhs=qkT[bp:bp + HD, hp, sb * QSB + qo:(sb + 1) * QSB],
                        start=True,
                        stop=True,
                    )
                pt = pt_pool.tile([P, 2, QSB], BF16, tag="pt")
                nc.scalar.activation(
                    pt[:, :, qo:], s2c[:, :, qo:],
                    mybir.ActivationFunctionType.Exp,
                    scale=SCALE,
                )
                if c >= 4 * sb:
                    for j in range(2):
                        # triangle at the causal diagonal block (DVE)
                        nc.vector.tensor_tensor(
                            pt[:, j, qo:qo + P],
                            pt[:, j, qo:qo + P],
                            tri[:],
                            mybir.AluOpType.mult,
                        )
                return (hp, c, qo, pt)

            def do_av(state):
                hp, c, qo, pt = state
                heads = (2 * hp, 2 * hp + 1)
                if c == 0:
                    # allocated here (not at issue time) so the previous
                    # head-pair's normalize reads are issued before the bank
                    # is recycled
                    ys[hp] = [
                        ps_y.tile([P, QSB], F32, tag="ps_y",
                                  name=f"yps{hp}_{i}")
                        for i in range(2)
                    ]
                for j, (h, y_ps) in enumerate(zip(heads, ys[hp])):
                    nc.tensor.matmul(
                        y_ps[0:HD + 1, qo:],
                        lhsT=v_sb[:, c, h, :],
                        rhs=pt[:, j, qo:],
                        start=(c == 0),
                        stop=(c == nch - 1),
                    )

            def normalize(hp, tail):
                # normalize: DVE copies only -- ACT copies would delay the
                # latency-critical exp chain on the ACT queue; the
                # reciprocal chain runs off base-partition-0 staging
                # (reciprocal_approx_fast reads garbage from nonzero bases).
                heads = (2 * hp, 2 * hp + 1)
                if tail:
                    # shortest DVE chain first (den -> recip -> bf16 cast);
                    # yb copies only feed the final mults, so they queue
                    # after. The sb3 out-proj prestart + reserved fillers
                    # keep the PE (and its p-state) hot throughout; the
                    # 1/den broadcast is a bf16 PE outer product.
                    dens = []
                    for j, y_ps in enumerate(ys[hp]):
                        den = r_pool.tile([1, QSB], F32, tag="den",
                                          name=f"den{j}")
                        nc.vector.tensor_copy(den[:], y_ps[HD:HD + 1, :])
                        dens.append(den)
                    if pre_norm_hook is not None:
                        pre_norm_hook()
                    rbs = []
                    for j in range(2):
                        r = r_pool.tile([1, QSB], F32, tag="r", name=f"r{j}")
                        nc.vector.reciprocal_approx_fast(r[:], dens[j][:])
                        rb = r_pool.tile([1, QSB], BF16, tag="rb",
                                         name=f"rb{j}")
                        nc.vector.tensor_copy(rb[:], r[:])
                        rbs.append(rb)
                    pop_fillers(2)
                    ybs = []
                    for j, y_ps in enumerate(ys[hp]):
                        yb = r_pool.tile([HD, QSB], F32, tag="yb",
                                         name=f"yb{j}")
                        nc.vector.tensor_copy(yb[:], y_ps[0:HD, :])
                        ybs.append(yb)
                    pop_fillers(2)
                    for j, (h, yb) in enumerate(zip(heads, ybs)):
                        bp = (h % 2) * HD
                        rbc_ps = ps_y.tile([P, QSB], F32, tag="ps_y",
                                           name=f"rbcps{j}")
                        nc.tensor.matmul(rbc_ps[0:HD, :], lhsT=ones64[:],
                                         rhs=rbs[j][:], start=True, stop=True)
                        nc.vector.tensor_tensor(
                            ySb[bp:bp + HD, hp, :], yb[:], rbc_ps[0:HD, :],
                            mybir.AluOpType.mult,
                        )
                    return
                ybs = []
                for j, y_ps in enumerate(ys[hp]):
                    den = r_pool.tile([1, QSB], F32, tag="den", name=f"den{j}")
                    nc.vector.tensor_copy(den[:], y_ps[HD:HD + 1, :])
                    yb = r_pool.tile([HD, QSB], F32, tag="yb", name=f"yb{j}")
                    nc.vector.tensor_copy(yb[:], y_ps[0:HD, :])
                    r = r_pool.tile([1, QSB], F32, tag="r", name=f"r{j}")
                    nc.vector.reciprocal_approx_fast(r[:], den[:])
                    rbc = r_pool.tile([HD, QSB], F32, tag="rbc", name=f"rbc{j}")
                    nc.gpsimd.partition_broadcast(rbc[:], r[:])
                    ybs.append((yb, rbc))

                def norm_mults(hp=hp, ybs=ybs, heads=heads):
                    for (h, (yb, rbc)) in zip(heads, ybs):
                        bp = (h % 2) * HD
                        nc.vector.tensor_tensor(
                            ySb[bp:bp + HD, hp, :], yb[:], rbc[:],
                            mybir.AluOpType.mult,
                        )
                pending.append(norm_mults)

            def tags_for(hp_, c_):
                return (("qk", sb, hp_), ("qk", c_ // 4, 4 + hp_),
                        ("v", c_))

            stream = [(hp, c) for hp in range(NH // 2) for c in range(nch)]
            state = issue_chunk(*stream[0])
            for idx, (hp, c) in enumerate(stream):
                nxt = (issue_chunk(*stream[idx + 1])
                       if idx + 1 < len(stream) else None)
                # pacing: pre-pop toward tags needed 4 iterations out
                # (capped, so ensure() never has to burst), else a per-sb
                # background drain -- early superblocks have cheap exps and
                # no ACT slack for extra PE work, late ones have plenty.
                la = stream[min(idx + 4, len(stream) - 1)]
                need = [t for t in tags_for(*la) if t not in done_tags]
                pops = 0
                while need and pops < 2 and fillers:
                    _run_filler()
                    pops += 1
                    need = [t for t in need if t not in done_tags]
                if (pops == 0 and idx % 4 != 3
                        and len(fillers) > tail_reserve):
                    pop_fillers(1)
                if c == 2 and pending:
                    pending.pop(0)()
                do_av(state)
                if c == nch - 1:
                    normalize(hp, tail_reserve and hp == NH // 2 - 1)
                state = nxt

        # ---- phase schedule ----
        # P0 (inline): only what sb0/hp0 needs up front -- q rc0, k rc4,
        # v chunks 0-1. Everything else is demand-popped via `ensure`.
        for tag, fn in (qk_rc(0, 0, alt=True) + qk_rc(0, 4, alt=True)
                        + v_sub(0, 0, alt=True) + v_sub(0, 1)):
            fn()
            if tag is not None:
                done_tags.add(tag)
        # rest of stripe 0 in sb0's demand order, then stripe 1.
        fillers += v_sub(0, 2) + v_sub(0, 3)
        for hp in range(1, 4):
            fillers += qk_rc(0, hp) + qk_rc(0, 4 + hp)

        # sb0 ||| stripe1. No inter-sb filler flushes: the deque carries
        # over so the ACT pipeline never starves while the PE bursts.
        load_stripe(1)
        fillers += stripe_fillers(1)
        attn_sb(0, bg_period=0)

        # sb1 ||| stripe2 + q3 (pulled early) + out0
        load_stripe(2)
        load_stripe(3)
        fillers += stripe_fillers(2)
        for rc in range(4):               # q rows of stripe 3
            fillers += qk_rc(3, rc)
        for sub in range(4):
            fillers += out_unit(0, sub)
        attn_sb(1, bg_period=4)

        # sb2 ||| k3 + out1
        for rc in range(4, 8):            # k rows of stripe 3
            fillers += qk_rc(3, rc)
        for sub in range(4):
            fillers += out_unit(1, sub)
        attn_sb(2, bg_period=2)

        # sb3 ||| v3 + out2; a few closures held back to keep the PE warm
        # through the last normalize so the final out-proj runs at max clock
        for sub in range(4):
            fillers += v_sub(3, sub)
        for sub in range(4):
            fillers += out_unit(2, sub)

        # sb3 out-proj row 0: contraction chunks 0-2 pre-start inside the
        # tail normalize (they only read head-pairs 0-2); the cc3 finisher
        # and rows 1-3 run after. The pre-started halves live in ps_sc --
        # free once the last exp retires, and never touched by fillers.
        o3_cells = {}

        def out3_prestart():
            ySb = ySbs[3]
            for nt in range(2):
                ps = ps_sc.tile([P, 512], F32, tag="ps_sc",
                                name=f"o3s{nt}")
                o3_cells[nt] = ps
                for cc in range(3):
                    nc.tensor.matmul(
                        ps[:],
                        lhsT=ySb[:, cc, 0:P],
                        rhs=wout_sb[:, cc, nt * 512:(nt + 1) * 512],
                        start=(cc == 0),
                        stop=False,
                        skip_group_check=True,
                    )

        attn_sb(3, tail_reserve=6, pre_norm_hook=out3_prestart, bg_period=1)
        flush_fillers()
        for fn in pending:
            fn()
        pending.clear()

        # tail: finish sb3 out-proj row 0, then rows 1-3
        ySb3 = ySbs[3]
        o3_t = o_pool.tile([P, 2, 512], F32, tag="osb", name="o3t")
        for nt in range(2):
            nc.tensor.matmul(
                o3_cells[nt][:],
                lhsT=ySb3[:, 3, 0:P],
                rhs=wout_sb[:, 3, nt * 512:(nt + 1) * 512],
                start=False,
                stop=True,
                skip_group_check=True,
            )
            nc.vector.tensor_copy(o3_t[:, nt, :], o3_cells[nt][:])
        nc.sync.dma_start(
            out[3 * QSB:3 * QSB + P, :], o3_t.rearrange("p a b -> p (a b)")
        )
        for sub in range(1, 4):
            for _tag, fn in out_unit(3, sub):
                fn()


_NC_CACHE = None


def _build_program():
    global _NC_CACHE
    if _NC_CACHE is not None:
        return _NC_CACHE
    nc = bacc.Bacc("TRN2", target_bir_lowering=False, debug=False)
    xT = nc.dram_tensor("xT", [D, S], BF16, kind="ExternalInput").ap()
    w_qk = nc.dram_tensor("w_qk", [D, 1024], BF16, kind="ExternalInput").ap()
    w_v = nc.dram_tensor("w_v", [D, 512], BF16, kind="ExternalInput").ap()
    w_out = nc.dram_tensor("w_out", [512, D], BF16, kind="ExternalInput").ap()
    out = nc.dram_tensor("out", [S, D], F32, kind="ExternalOutput").ap()
    with tile.TileContext(nc) as tc:
        _attention_kernel(tc, out, xT, w_qk, w_v, w_out)
    nc.compile()
    _NC_CACHE = nc
    return nc


def make_in_maps(x, W_qkv, W_out):
    import ml_dtypes

    bf16 = ml_dtypes.bfloat16
    x = np.ascontiguousarray(np.asarray(x, dtype=np.float32))
    W_qkv = np.asarray(W_qkv, dtype=np.float32)
    W_out = np.asarray(W_out, dtype=np.float32)
    in_maps = []
    for c in range(8):
        b, g = divmod(c, 2)
        lo = 512 * g
        cols = np.arange(lo, lo + 512)
        in_maps.append({
            "xT": np.ascontiguousarray(x[b].T).astype(bf16),
            "w_qk": np.ascontiguousarray(
                np.concatenate([W_qkv[:, cols], W_qkv[:, D + cols]], axis=1)
            ).astype(bf16),
            "w_v": np.ascontiguousarray(W_qkv[:, 2 * D + cols]).astype(bf16),
            "w_out": np.ascontiguousarray(W_out[cols, :]).astype(bf16),
        })
    return in_maps


def combine_outputs(results):
    # results: list of 8 dicts with "out" [S, D]; core c = 2*b + g
    return np.stack(
        [results[2 * b]["out"] + results[2 * b + 1]["out"] for b in range(B)]
    ).astype(np.float32)


def kernel(x, W_qkv, W_out):
    nc = _build_program()
    in_maps = make_in_maps(x, W_qkv, W_out)
    res = run_bass_kernel_spmd(nc, in_maps, core_ids=list(range(8)))
    return combine_outputs(res.results)


if __name__ == "__main__":
    # smoke test against a local numpy reference
    rng = np.random.default_rng(0)
    x = rng.standard_normal((B, S, D), dtype=np.float32)
    W_qkv = (rng.standard_normal((D, 3 * D)) * 0.02).astype(np.float32)
    W_out = (rng.standard_normal((D, D)) * 0.02).astype(np.float32)
    out = kernel(x, W_qkv, W_out)
    print("out", out.shape, out.dtype, float(np.abs(out).mean()))



# revision 78
# speedup vs baseline: 1.0138x; 1.0138x over previous
"""Causal self-attention (B=4, S=2048, D=1024, H=16, hd=64) on 8 TRN2 NeuronCores.

Sharding: batch 4-way x head-group 2-way. Core c = 2*b + g handles batch b and
heads [8g, 8g+8). Each core computes the QKV projection for its heads, causal
flash-style attention, and a partial output projection; the host sums the two
head-group partials per batch.

v3 schedule -- a software-pipelined per-chunk stream:
  - Attention runs over a flattened (head-pair, kv-chunk) stream with a
    one-chunk lookahead: QK(c+1) is issued on the PE queue BEFORE AV(c), so
    the next chunk's scores stream while this chunk's exp runs on ACT,
    including across head-pair boundaries. Score tiles are per-chunk
    [P, head, QSB] (slab = head, adjacent psum banks), freed by their own
    single two-head exp -- 2 score bufs suffice for the lookahead, and exp
    never strictly alternates with QK.
  - A global deque of ~0.85us filler granules (projection half-contraction
    pieces, deferred out-projection halves) keeps the PE saturated and its
    p-state at max. Fillers are appended in each superblock's demand order
    and tagged by the data they publish (qkT rows, v chunks); consumers
    demand-pop via ensure() so correctness never depends on pacing, while a
    capped 4-iteration lookahead pre-pop plus a 3-of-4 background drain
    keeps the deque smooth.
  - Only the first head-pair's stripe-0 pieces run inline at startup; the
    rest demand-pop behind the DMA stream.
  - The sb3 out-projection's first 3 contraction chunks pre-start inside the
    final normalize (they only need head-pairs 0-2), covering the
    reciprocal chain so the tail runs at full clock.
Attention per (head-pair, chunk): S^T = K.Q^T lands P^T via exp() in the
layout the P^T.V matmul wants; a ones-column in V yields softmax denominators
(row 64). Head pairs sit on disjoint PE row halves so their QK matmuls stream
concurrently (PE row-group tiling). No running-max: scores are bounded, exp
stays finite in fp32.
"""

import sys

for _p in ("/opt/trn_rl_repo",):
    if _p not in sys.path:
        sys.path.insert(0, _p)

from contextlib import ExitStack

import numpy as np

import concourse.bass as bass
import concourse.mybir as mybir
import concourse.tile as tile
from concourse import bacc
from concourse.bass_utils import run_bass_kernel_spmd

F32 = mybir.dt.float32
BF16 = mybir.dt.bfloat16
P = 128
B, S, D = 4, 2048, 1024
HD = 64          # head dim
NH = 8           # heads per core
KO = D // P      # 8 contraction chunks for the projections
QSB = 512        # q superblock (matmul free dim)
N_SB = S // QSB  # 4
N_SC = S // P    # 16 kv chunks
PSTRIPE = 512    # s-stripe for the projection phase
SCALE = 0.125    # 1/sqrt(64)


def _attention_kernel(tc, out, xT, w_qk, w_v, w_out):
    nc = tc.nc
    with ExitStack() as ctx:
        const_pool = ctx.enter_context(tc.tile_pool(name="const", bufs=1))
        qkT_pool = ctx.enter_context(tc.tile_pool(name="qkT", bufs=1))
        v_pool = ctx.enter_context(tc.tile_pool(name="vsb", bufs=1))
        wqk_pool = ctx.enter_context(tc.tile_pool(name="wqk", bufs=1))
        wv_pool = ctx.enter_context(tc.tile_pool(name="wv", bufs=1))
        wout_pool = ctx.enter_context(tc.tile_pool(name="wout", bufs=1))
        xt_pool = ctx.enter_context(tc.tile_pool(name="xt", bufs=3))
        pt_pool = ctx.enter_context(tc.tile_pool(name="pt", bufs=6))
        y_pool = ctx.enter_context(tc.tile_pool(name="ysb", bufs=2))
        r_pool = ctx.enter_context(tc.tile_pool(name="recip", bufs=6))
        o_pool = ctx.enter_context(tc.tile_pool(name="osb", bufs=3))
        # PSUM: scores 2x2 banks + work(proj/out) 2x1 + y accum 2x1 = 8 banks
        ps_sc = ctx.enter_context(tc.tile_pool(name="ps_sc", bufs=2, space="PSUM"))
        ps_wk = ctx.enter_context(tc.tile_pool(name="ps_wk", bufs=2, space="PSUM"))
        ps_y = ctx.enter_context(tc.tile_pool(name="ps_y", bufs=2, space="PSUM"))

        # 128x128 triangle for the diagonal block (transposed layout):
        # tri[i, j] = 1 if j >= i else 0
        tri = const_pool.tile([P, P], BF16, tag="tri")
        nc.gpsimd.memset(tri[:], 1.0)
        ones64 = const_pool.tile([1, HD], BF16, tag="ones64")
        nc.gpsimd.memset(ones64[:], 1.0)
        nc.gpsimd.affine_select(
            out=tri[:],
            in_=tri[:],
            compare_op=mybir.AluOpType.is_ge,
            fill=0.0,
            base=0,
            channel_multiplier=-1,
            pattern=[[1, P]],
        )

        # p-state warmup: junk matmuls with no DMA dependencies keep the
        # PE busy (and its clock ramping) while the first inputs land.
        warm_w = const_pool.tile([P, P], BF16, tag="warmw")
        nc.gpsimd.memset(warm_w[:], 1.0)
        warm_x = const_pool.tile([P, 512], BF16, tag="warmx")
        nc.gpsimd.memset(warm_x[:], 1.0)
        warm_ps = ps_y.tile([P, 512], F32, tag="ps_y", name="warmps")
        # q^T/k^T store: row-chunk rc<4 holds q rows, rc>=4 holds k rows.
        # Head h lives at partitions 64*(h%2)..+64 of row-chunk h//2 (+4 for k).
        qkT = qkT_pool.tile([P, 8, S], BF16)
        for _w in range(10):
            nc.tensor.matmul(warm_ps[:], lhsT=warm_w[:], rhs=warm_x[:],
                             start=True, stop=True)
        # V store: [s-partition, kv-chunk, head, hd+1]; last col is ones for the
        # softmax denominator.
        v_sb = v_pool.tile([P, N_SC, NH, HD + 1], BF16)
        nc.gpsimd.memset(v_sb[:, :, :, HD], 1.0)

        # Startup loads: wqk rides full-width rows (2KB/partition DMA lines
        # run ~1.7x faster than the 1KB halves), so q+k weights land in
        # ~5.7us instead of 9.4.
        wqk_sb = wqk_pool.tile([P, KO, 2 * 512], BF16)
        xts = [None] * 4
        xts[0] = xt_pool.tile([P, KO, PSTRIPE], BF16, tag="xt", name="xt0")
        for ko in range(KO):
            nc.sync.dma_start(
                wqk_sb[:, ko, :],
                w_qk[ko * P:(ko + 1) * P, :],
            )
        wv_sb = wv_pool.tile([P, KO, 512], BF16)
        for ko in range(KO):
            nc.sync.dma_start(xts[0][:, ko, :],
                              xT[ko * P:(ko + 1) * P, 0:PSTRIPE])
        for ko in range(KO):
            nc.sync.dma_start(wv_sb[:, ko, :], w_v[ko * P:(ko + 1) * P, :])
        wout_sb = wout_pool.tile([P, 4, D], BF16)
        for co in range(4):
            nc.sync.dma_start(wout_sb[:, co, :], w_out[co * P:(co + 1) * P, :])

        def load_stripe(st):
            xt = xt_pool.tile([P, KO, PSTRIPE], BF16, tag="xt", name=f"xt{st}")
            for ko in range(KO):
                nc.sync.dma_start(
                    xt[:, ko, :],
                    xT[ko * P:(ko + 1) * P, st * PSTRIPE:(st + 1) * PSTRIPE],
                )
            xts[st] = xt

        # ---- filler units: closures of ~0.9-1.9us of PE work ----

        def qk_rc(st, rc, alt=False):
            """q^T/k^T rows for row-chunk rc of stripe st: two ~0.85us
            closures (half-contraction granules) so fillers never overshoot
            the per-iteration PE slack."""
            cell = {}

            def half(koh):
                if koh == 0:
                    pool, tg = (ps_y, "ps_y") if alt else (ps_wk, "ps_wk")
                    cell[0] = pool.tile(
                        [P, PSTRIPE], F32, tag=tg, name=f"pqk{st}_{rc}"
                    )
                ps = cell[0]
                for ko in range(4 * koh, 4 * koh + 4):
                    nc.tensor.matmul(
                        ps[:],
                        lhsT=wqk_sb[:, ko, rc * P:(rc + 1) * P],
                        rhs=xts[st][:, ko, :],
                        start=(ko == 0),
                        stop=(ko == KO - 1),
                    )
                if koh == 1:
                    nc.vector.tensor_copy(
                        qkT[:, rc, st * PSTRIPE:(st + 1) * PSTRIPE], ps[:]
                    )

            return [(None, lambda: half(0)),
                    (("qk", st, rc), lambda: half(1))]

        def v_sub(st, sub, alt=False):
            """V rows for s-chunk 4*st+sub: two ~0.85us closures."""
            cell = {}

            def half(koh):
                if koh == 0:
                    pool, tg = (ps_y, "ps_y") if alt else (ps_wk, "ps_wk")
                    cell[0] = pool.tile(
                        [P, NH * HD], F32, tag=tg, name=f"pv{st}_{sub}"
                    )
                ps = cell[0]
                for ko in range(4 * koh, 4 * koh + 4):
                    nc.tensor.matmul(
                        ps[:],
                        lhsT=xts[st][:, ko, sub * P:(sub + 1) * P],
                        rhs=wv_sb[:, ko, :],
                        start=(ko == 0),
                        stop=(ko == KO - 1),
                    )
                if koh == 1:
                    sc = st * (PSTRIPE // P) + sub
                    nc.vector.tensor_copy(
                        v_sb[:, sc, :, 0:HD],
                        ps.rearrange("p (h e) -> p h e", h=NH),
                    )

            return [(None, lambda: half(0)),
                    (("v", st * 4 + sub), lambda: half(1))]

        def stripe_fillers(st):
            """All of stripe st's proj pieces in the demand order of
            superblock st's attention: q/k rows for hp0 first, then the v
            chunks (needed from iteration 4*hp0+...), then later head-pairs'
            rows interleaved q,k."""
            out = []
            out += qk_rc(st, 0) + qk_rc(st, 4)
            for sub in range(4):
                out += v_sub(st, sub)
            for hp in range(1, 4):
                out += qk_rc(st, hp) + qk_rc(st, 4 + hp)
            return out

        ySbs = [None] * N_SB

        def out_unit(sb, sub):
            """Output projection for s-rows sb*512+sub*128..+128. Two closures."""
            cell = {}

            def half(nt):
                ps = ps_wk.tile([P, 512], F32, tag="ps_wk", name=f"ops{sb}_{sub}_{nt}")
                ySb = ySbs[sb]
                for cc in range(4):
                    nc.tensor.matmul(
                        ps[:],
                        lhsT=ySb[:, cc, sub * P:(sub + 1) * P],
                        rhs=wout_sb[:, cc, nt * 512:(nt + 1) * 512],
                        start=(cc == 0),
                        stop=(cc == 3),
                    )
                if nt == 0:
                    cell["o_t"] = o_pool.tile([P, 2, 512], F32, tag="osb", name=f"ot{sb}_{sub}")
                o_t = cell["o_t"]
                nc.vector.tensor_copy(o_t[:, nt, :], ps[:])
                if nt == 1:
                    row = (sb * (QSB // P) + sub) * P
                    nc.sync.dma_start(
                        out[row:row + P, :], o_t.rearrange("p a b -> p (a b)")
                    )

            return [(None, lambda: half(0)), (None, lambda: half(1))]

        # global filler deque of (tag, closure); tags mark closures that
        # publish data (qkT rows, v chunks) so consumers can demand-pop
        # them before issuing reads -- correctness no longer depends on
        # pop pacing.
        fillers = []
        done_tags = set()

        def _run_filler():
            tag, fn = fillers.pop(0)
            fn()
            if tag is not None:
                done_tags.add(tag)
            return tag

        def pop_fillers(n):
            for _ in range(n):
                if fillers:
                    _run_filler()

        def flush_fillers():
            while fillers:
                _run_filler()

        def ensure(tag):
            if tag in done_tags:
                return
            while fillers:
                if _run_filler() == tag:
                    return

        pending = []   # deferred normalize multiplies (previous head-pair)

        def attn_sb(sb, tail_reserve=0, pre_norm_hook=None, bg_period=2):
            ySb = y_pool.tile([P, 4, QSB], BF16, tag="ysb", name=f"ysb{sb}")
            ySbs[sb] = ySb
            nch = 4 * (sb + 1)
            ys = {}

            # One-chunk software pipeline over a flattened (head-pair, chunk)
            # stream: QK(next) is issued on the PE queue BEFORE AV(cur), so
            # the next chunk's scores stream while this chunk's exp runs on
            # ACT -- including across head-pair boundaries. A chunk's score
            # tile [P, head, QSB] frees as soon as its own (single, two-head)
            # exp retires, so 2 score bufs suffice for the lookahead.
            def issue_chunk(hp, c):
                heads = (2 * hp, 2 * hp + 1)
                rc_k = 4 + hp
                qo = P * max(0, c - 4 * sb)
                # demand-pop the fillers that publish this chunk's inputs
                ensure(("qk", sb, hp))        # q rows, stripe sb
                ensure(("qk", c // 4, rc_k))  # k rows, chunk stripe
                ensure(("v", c))              # v rows (read 1 iter later)
                s2c = ps_sc.tile([P, 2, QSB], F32, tag="ps_sc",
                                 name=f"s2c{c % 2}")
                for j, h in enumerate(heads):
                    bp = (h % 2) * HD
                    nc.tensor.matmul(
                        s2c[:, j, qo:],
                        lhsT=qkT[bp:bp + HD, rc_k, c * P:(c + 1) * P],
                        rhs=qkT[bp:bp + HD, hp, sb * QSB + qo:(sb + 1) * QSB],
                        start=True,
                        stop=True,
                    )
                pt = pt_pool.tile([P, 2, QSB], BF16, tag="pt")
                nc.scalar.activation(
                    pt[:, :, qo:], s2c[:, :, qo:],
                    mybir.ActivationFunctionType.Exp,
                    scale=SCALE,
                )
                if c >= 4 * sb:
                    for j in range(2):
                        # triangle at the causal diagonal block (DVE)
                        nc.vector.tensor_tensor(
                            pt[:, j, qo:qo + P],
                            pt[:, j, qo:qo + P],
                            tri[:],
                            mybir.AluOpType.mult,
                        )
                return (hp, c, qo, pt)

            def do_av(state):
                hp, c, qo, pt = state
                heads = (2 * hp, 2 * hp + 1)
                if c == 0:
                    # allocated here (not at issue time) so the previous
                    # head-pair's normalize reads are issued before the bank
                    # is recycled
                    ys[hp] = [
                        ps_y.tile([P, QSB], F32, tag="ps_y",
                                  name=f"yps{hp}_{i}")
                        for i in range(2)
                    ]
                for j, (h, y_ps) in enumerate(zip(heads, ys[hp])):
                    nc.tensor.matmul(
                        y_ps[0:HD + 1, qo:],
                        lhsT=v_sb[:, c, h, :],
                        rhs=pt[:, j, qo:],
                        start=(c == 0),
                        stop=(c == nch - 1),
                    )

            def normalize(hp, tail):
                # normalize: DVE copies only -- ACT copies would delay the
                # latency-critical exp chain on the ACT queue; the
                # reciprocal chain runs off base-partition-0 staging
                # (reciprocal_approx_fast reads garbage from nonzero bases).
                heads = (2 * hp, 2 * hp + 1)
                if tail:
                    # shortest DVE chain first (den -> recip -> bf16 cast);
                    # yb copies only feed the final mults, so they queue
                    # after. The sb3 out-proj prestart + reserved fillers
                    # keep the PE (and its p-state) hot throughout; the
                    # 1/den broadcast is a bf16 PE outer product.
                    dens = []
                    for j, y_ps in enumerate(ys[hp]):
                        den = r_pool.tile([1, QSB], F32, tag="den",
                                          name=f"den{j}")
                        nc.vector.tensor_copy(den[:], y_ps[HD:HD + 1, :])
                        dens.append(den)
                    if pre_norm_hook is not None:
                        pre_norm_hook()
                    rbs = []
                    for j in range(2):
                        r = r_pool.tile([1, QSB], F32, tag="r", name=f"r{j}")
                        nc.vector.reciprocal_approx_fast(r[:], dens[j][:])
                        rb = r_pool.tile([1, QSB], BF16, tag="rb",
                                         name=f"rb{j}")
                        nc.vector.tensor_copy(rb[:], r[:])
                        rbs.append(rb)
                    pop_fillers(2)
                    ybs = []
                    for j, y_ps in enumerate(ys[hp]):
                        yb = r_pool.tile([HD, QSB], F32, tag="yb",
                                         name=f"yb{j}")
                        nc.vector.tensor_copy(yb[:], y_ps[0:HD, :])
                        ybs.append(yb)
                    pop_fillers(2)
                    for j, (h, yb) in enumerate(zip(heads, ybs)):
                        bp = (h % 2) * HD
                        rbc_ps = ps_y.tile([P, QSB], F32, tag="ps_y",
                                           name=f"rbcps{j}")
                        nc.tensor.matmul(rbc_ps[0:HD, :], lhsT=ones64[:],
                                         rhs=rbs[j][:], start=True, stop=True)
                        nc.vector.tensor_tensor(
                            ySb[bp:bp + HD, hp, :], yb[:], rbc_ps[0:HD, :],
                            mybir.AluOpType.mult,
                        )
                    return
                ybs = []
                for j, y_ps in enumerate(ys[hp]):
                    den = r_pool.tile([1, QSB], F32, tag="den", name=f"den{j}")
                    nc.vector.tensor_copy(den[:], y_ps[HD:HD + 1, :])
                    yb = r_pool.tile([HD, QSB], F32, tag="yb", name=f"yb{j}")
                    nc.vector.tensor_copy(yb[:], y_ps[0:HD, :])
                    r = r_pool.tile([1, QSB], F32, tag="r", name=f"r{j}")
                    nc.vector.reciprocal_approx_fast(r[:], den[:])
                    rbc = r_pool.tile([HD, QSB], F32, tag="rbc", name=f"rbc{j}")
                    nc.gpsimd.partition_broadcast(rbc[:], r[:])
                    ybs.append((yb, rbc))

                def norm_mults(hp=hp, ybs=ybs, heads=heads):
                    for (h, (yb, rbc)) in zip(heads, ybs):
                        bp = (h % 2) * HD
                        nc.vector.tensor_tensor(
                            ySb[bp:bp + HD, hp, :], yb[:], rbc[:],
                            mybir.AluOpType.mult,
                        )
                pending.append(norm_mults)

            def tags_for(hp_, c_):
                return (("qk", sb, hp_), ("qk", c_ // 4, 4 + hp_),
                        ("v", c_))

            stream = [(hp, c) for hp in range(NH // 2) for c in range(nch)]
            state = issue_chunk(*stream[0])
            for idx, (hp, c) in enumerate(stream):
                nxt = (issue_chunk(*stream[idx + 1])
                       if idx + 1 < len(stream) else None)
                # pacing: pre-pop toward tags needed 4 iterations out
                # (capped, so ensure() never has to burst), else a per-sb
                # background drain -- early superblocks have cheap exps and
                # no ACT slack for extra PE work, late ones have plenty.
                la = stream[min(idx + 4, len(stream) - 1)]
                need = [t for t in tags_for(*la) if t not in done_tags]
                pops = 0
                while need and pops < 2 and fillers:
                    _run_filler()
                    pops += 1
                    need = [t for t in need if t not in done_tags]
                if (pops == 0 and idx % 4 != 3
                        and len(fillers) > tail_reserve):
                    pop_fillers(1)
                if c == 2 and pending:
                    pending.pop(0)()
                do_av(state)
                if c == nch - 1:
                    normalize(hp, tail_reserve and hp == NH // 2 - 1)
                state = nxt

        # ---- phase schedule ----
        # P0 (inline): only what sb0/hp0 needs up front -- q rc0, k rc4,
        # v chunks 0-1. Everything else is demand-popped via `ensure`.
        for tag, fn in (qk_rc(0, 0, alt=True) + qk_rc(0, 4, alt=True)
                        + v_sub(0, 0, alt=True) + v_sub(0, 1)):
            fn()
            if tag is not None:
                done_tags.add(tag)
        # rest of stripe 0 in sb0's demand order, then stripe 1.
        fillers += v_sub(0, 2) + v_sub(0, 3)
        for hp in range(1, 4):
            fillers += qk_rc(0, hp) + qk_rc(0, 4 + hp)

        # sb0 ||| stripe1. No inter-sb filler flushes: the deque carries
        # over so the ACT pipeline never starves while the PE bursts.
        load_stripe(1)
        fillers += stripe_fillers(1)
        attn_sb(0, bg_period=0)

        # sb1 ||| stripe2 + q3 (pulled early) + out0
        load_stripe(2)
        load_stripe(3)
        fillers += stripe_fillers(2)
        for rc in range(4):               # q rows of stripe 3
            fillers += qk_rc(3, rc)
        for sub in range(4):
            fillers += out_unit(0, sub)
        attn_sb(1, bg_period=4)

        # sb2 ||| k3 + out1
        for rc in range(4, 8):            # k rows of stripe 3
            fillers += qk_rc(3, rc)
        for sub in range(4):
            fillers += out_unit(1, sub)
        attn_sb(2, bg_period=2)

        # sb3 ||| v3 + out2; a few closures held back to keep the PE warm
        # through the last normalize so the final out-proj runs at max clock
        for sub in range(4):
            fillers += v_sub(3, sub)
        for sub in range(4):
            fillers += out_unit(2, sub)

        # sb3 out-proj row 0: contraction chunks 0-2 pre-start inside the
        # tail normalize (they only read head-pairs 0-2); the cc3 finisher
        # and rows 1-3 run after. The pre-started halves live in ps_sc --
        # free once the last exp retires, and never touched by fillers.
        o3_cells = {}

        def out3_prestart():
            ySb = ySbs[3]
            for nt in range(2):
                ps = ps_sc.tile([P, 512], F32, tag="ps_sc",
                                name=f"o3s{nt}")
                o3_cells[nt] = ps
                for cc in range(3):
                    nc.tensor.matmul(
                        ps[:],
                        lhsT=ySb[:, cc, 0:P],
                        rhs=wout_sb[:, cc, nt * 512:(nt + 1) * 512],
                        start=(cc == 0),
                        stop=False,
                        skip_group_check=True,
                    )

        attn_sb(3, tail_reserve=4, pre_norm_hook=out3_prestart, bg_period=1)
        flush_fillers()
        for fn in pending:
            fn()
        pending.clear()

        # tail: finish sb3 out-proj row 0, then rows 1-3
        ySb3 = ySbs[3]
        o3_t = o_pool.tile([P, 2, 512], F32, tag="osb", name="o3t")
        for nt in range(2):
            nc.tensor.matmul(
                o3_cells[nt][:],
                lhsT=ySb3[:, 3, 0:P],
                rhs=wout_sb[:, 3, nt * 512:(nt + 1) * 512],
                start=False,
                stop=True,
                skip_group_check=True,
            )
            nc.vector.tensor_copy(o3_t[:, nt, :], o3_cells[nt][:])
        nc.sync.dma_start(
            out[3 * QSB:3 * QSB + P, :], o3_t.rearrange("p a b -> p (a b)")
        )
        for sub in range(1, 4):
            for _tag, fn in out_unit(3, sub):
                fn()


_NC_CACHE = None


def _build_program():
    global _NC_CACHE
    if _NC_CACHE is not None:
        return _NC_CACHE
    nc = bacc.Bacc("TRN2", target_bir_lowering=False, debug=False)
    xT = nc.dram_tensor("xT", [D, S], BF16, kind="ExternalInput").ap()
    w_qk = nc.dram_tensor("w_qk", [D, 1024], BF16, kind="ExternalInput").ap()
    w_v = nc.dram_tensor("w_v", [D, 512], BF16, kind="ExternalInput").ap()
    w_out = nc.dram_tensor("w_out", [512, D], BF16, kind="ExternalInput").ap()
    out = nc.dram_tensor("out", [S, D], F32, kind="ExternalOutput").ap()
    with tile.TileContext(nc) as tc:
        _attention_kernel(tc, out, xT, w_qk, w_v, w_out)
    nc.compile()
    _NC_CACHE = nc
    return nc


def make_in_maps(x, W_qkv, W_out):
    import ml_dtypes

    bf16 = ml_dtypes.bfloat16
    x = np.ascontiguousarray(np.asarray(x, dtype=np.float32))
    W_qkv = np.asarray(W_qkv, dtype=np.float32)
    W_out = np.asarray(W_out, dtype=np.float32)
    in_maps = []
    for c in range(8):
        b, g = divmod(c, 2)
        lo = 512 * g
        cols = np.arange(lo, lo + 512)
        in_maps.append({
            "xT": np.ascontiguousarray(x[b].T).astype(bf16),
            "w_qk": np.ascontiguousarray(
                np.concatenate([W_qkv[:, cols], W_qkv[:, D + cols]], axis=1)
            ).astype(bf16),
            "w_v": np.ascontiguousarray(W_qkv[:, 2 * D + cols]).astype(bf16),
            "w_out": np.ascontiguousarray(W_out[cols, :]).astype(bf16),
        })
    return in_maps


def combine_outputs(results):
    # results: list of 8 dicts with "out" [S, D]; core c = 2*b + g
    return np.stack(
        [results[2 * b]["out"] + results[2 * b + 1]["out"] for b in range(B)]
    ).astype(np.float32)


def kernel(x, W_qkv, W_out):
    nc = _build_program()
    in_maps = make_in_maps(x, W_qkv, W_out)
    res = run_bass_kernel_spmd(nc, in_maps, core_ids=list(range(8)))
    return combine_outputs(res.results)


if __name__ == "__main__":
    # smoke test against a local numpy reference
    rng = np.random.default_rng(0)
    x = rng.standard_normal((B, S, D), dtype=np.float32)
    W_qkv = (rng.standard_normal((D, 3 * D)) * 0.02).astype(np.float32)
    W_out = (rng.standard_normal((D, D)) * 0.02).astype(np.float32)
    out = kernel(x, W_qkv, W_out)
    print("out", out.shape, out.dtype, float(np.abs(out).mean()))



# revision 79
# speedup vs baseline: 1.0144x; 1.0006x over previous
"""Causal self-attention (B=4, S=2048, D=1024, H=16, hd=64) on 8 TRN2 NeuronCores.

Sharding: batch 4-way x head-group 2-way. Core c = 2*b + g handles batch b and
heads [8g, 8g+8). Each core computes the QKV projection for its heads, causal
flash-style attention, and a partial output projection; the host sums the two
head-group partials per batch.

v3 schedule -- a software-pipelined per-chunk stream:
  - Attention runs over a flattened (head-pair, kv-chunk) stream with a
    one-chunk lookahead: QK(c+1) is issued on the PE queue BEFORE AV(c), so
    the next chunk's scores stream while this chunk's exp runs on ACT,
    including across head-pair boundaries. Score tiles are per-chunk
    [P, head, QSB] (slab = head, adjacent psum banks), freed by their own
    single two-head exp -- 2 score bufs suffice for the lookahead, and exp
    never strictly alternates with QK.
  - A global deque of ~0.85us filler granules (projection half-contraction
    pieces, deferred out-projection halves) keeps the PE saturated and its
    p-state at max. Fillers are appended in each superblock's demand order
    and tagged by the data they publish (qkT rows, v chunks); consumers
    demand-pop via ensure() so correctness never depends on pacing, while a
    capped 4-iteration lookahead pre-pop plus a 3-of-4 background drain
    keeps the deque smooth.
  - Only the first head-pair's stripe-0 pieces run inline at startup; the
    rest demand-pop behind the DMA stream.
  - The sb3 out-projection's first 3 contraction chunks pre-start inside the
    final normalize (they only need head-pairs 0-2), covering the
    reciprocal chain so the tail runs at full clock.
Attention per (head-pair, chunk): S^T = K.Q^T lands P^T via exp() in the
layout the P^T.V matmul wants; a ones-column in V yields softmax denominators
(row 64). Head pairs sit on disjoint PE row halves so their QK matmuls stream
concurrently (PE row-group tiling). No running-max: scores are bounded, exp
stays finite in fp32.
"""

import sys

for _p in ("/opt/trn_rl_repo",):
    if _p not in sys.path:
        sys.path.insert(0, _p)

from contextlib import ExitStack

import numpy as np

import concourse.bass as bass
import concourse.mybir as mybir
import concourse.tile as tile
from concourse import bacc
from concourse.bass_utils import run_bass_kernel_spmd

F32 = mybir.dt.float32
BF16 = mybir.dt.bfloat16
P = 128
B, S, D = 4, 2048, 1024
HD = 64          # head dim
NH = 8           # heads per core
KO = D // P      # 8 contraction chunks for the projections
QSB = 512        # q superblock (matmul free dim)
N_SB = S // QSB  # 4
N_SC = S // P    # 16 kv chunks
PSTRIPE = 512    # s-stripe for the projection phase
SCALE = 0.125    # 1/sqrt(64)


def _attention_kernel(tc, out, xT, w_qk, w_v, w_out):
    nc = tc.nc
    with ExitStack() as ctx:
        const_pool = ctx.enter_context(tc.tile_pool(name="const", bufs=1))
        qkT_pool = ctx.enter_context(tc.tile_pool(name="qkT", bufs=1))
        v_pool = ctx.enter_context(tc.tile_pool(name="vsb", bufs=1))
        wqk_pool = ctx.enter_context(tc.tile_pool(name="wqk", bufs=1))
        wv_pool = ctx.enter_context(tc.tile_pool(name="wv", bufs=1))
        wout_pool = ctx.enter_context(tc.tile_pool(name="wout", bufs=1))
        xt_pool = ctx.enter_context(tc.tile_pool(name="xt", bufs=3))
        pt_pool = ctx.enter_context(tc.tile_pool(name="pt", bufs=6))
        y_pool = ctx.enter_context(tc.tile_pool(name="ysb", bufs=2))
        r_pool = ctx.enter_context(tc.tile_pool(name="recip", bufs=6))
        o_pool = ctx.enter_context(tc.tile_pool(name="osb", bufs=3))
        # PSUM: scores 2x2 banks + work(proj/out) 2x1 + y accum 2x1 = 8 banks
        ps_sc = ctx.enter_context(tc.tile_pool(name="ps_sc", bufs=2, space="PSUM"))
        ps_wk = ctx.enter_context(tc.tile_pool(name="ps_wk", bufs=2, space="PSUM"))
        ps_y = ctx.enter_context(tc.tile_pool(name="ps_y", bufs=2, space="PSUM"))

        # 128x128 triangle for the diagonal block (transposed layout):
        # tri[i, j] = 1 if j >= i else 0
        tri = const_pool.tile([P, P], BF16, tag="tri")
        nc.gpsimd.memset(tri[:], 1.0)
        ones64 = const_pool.tile([1, HD], BF16, tag="ones64")
        nc.gpsimd.memset(ones64[:], 1.0)
        nc.gpsimd.affine_select(
            out=tri[:],
            in_=tri[:],
            compare_op=mybir.AluOpType.is_ge,
            fill=0.0,
            base=0,
            channel_multiplier=-1,
            pattern=[[1, P]],
        )

        # p-state warmup: junk matmuls with no DMA dependencies keep the
        # PE busy (and its clock ramping) while the first inputs land.
        warm_w = const_pool.tile([P, P], BF16, tag="warmw")
        nc.gpsimd.memset(warm_w[:], 1.0)
        warm_x = const_pool.tile([P, 512], BF16, tag="warmx")
        nc.gpsimd.memset(warm_x[:], 1.0)
        warm_ps = ps_y.tile([P, 512], F32, tag="ps_y", name="warmps")
        # q^T/k^T store: row-chunk rc<4 holds q rows, rc>=4 holds k rows.
        # Head h lives at partitions 64*(h%2)..+64 of row-chunk h//2 (+4 for k).
        qkT = qkT_pool.tile([P, 8, S], BF16)
        for _w in range(10):
            nc.tensor.matmul(warm_ps[:], lhsT=warm_w[:], rhs=warm_x[:],
                             start=True, stop=True)
        # V store: [s-partition, kv-chunk, head, hd+1]; last col is ones for the
        # softmax denominator.
        v_sb = v_pool.tile([P, N_SC, NH, HD + 1], BF16)
        nc.gpsimd.memset(v_sb[:, :, :, HD], 1.0)

        # Startup loads: wqk rides full-width rows (2KB/partition DMA lines
        # run ~1.7x faster than the 1KB halves), so q+k weights land in
        # ~5.7us instead of 9.4.
        wqk_sb = wqk_pool.tile([P, KO, 2 * 512], BF16)
        xts = [None] * 4
        xts[0] = xt_pool.tile([P, KO, PSTRIPE], BF16, tag="xt", name="xt0")
        for ko in range(KO):
            nc.sync.dma_start(
                wqk_sb[:, ko, :],
                w_qk[ko * P:(ko + 1) * P, :],
            )
        wv_sb = wv_pool.tile([P, KO, 512], BF16)
        for ko in range(KO):
            nc.sync.dma_start(xts[0][:, ko, :],
                              xT[ko * P:(ko + 1) * P, 0:PSTRIPE])
        for ko in range(KO):
            nc.sync.dma_start(wv_sb[:, ko, :], w_v[ko * P:(ko + 1) * P, :])
        wout_sb = wout_pool.tile([P, 4, D], BF16)
        for co in range(4):
            nc.sync.dma_start(wout_sb[:, co, :], w_out[co * P:(co + 1) * P, :])

        def load_stripe(st):
            xt = xt_pool.tile([P, KO, PSTRIPE], BF16, tag="xt", name=f"xt{st}")
            for ko in range(KO):
                nc.sync.dma_start(
                    xt[:, ko, :],
                    xT[ko * P:(ko + 1) * P, st * PSTRIPE:(st + 1) * PSTRIPE],
                )
            xts[st] = xt

        # ---- filler units: closures of ~0.9-1.9us of PE work ----

        def qk_rc(st, rc, alt=False):
            """q^T/k^T rows for row-chunk rc of stripe st: two ~0.85us
            closures (half-contraction granules) so fillers never overshoot
            the per-iteration PE slack."""
            cell = {}

            def half(koh):
                if koh == 0:
                    pool, tg = (ps_y, "ps_y") if alt else (ps_wk, "ps_wk")
                    cell[0] = pool.tile(
                        [P, PSTRIPE], F32, tag=tg, name=f"pqk{st}_{rc}"
                    )
                ps = cell[0]
                for ko in range(4 * koh, 4 * koh + 4):
                    nc.tensor.matmul(
                        ps[:],
                        lhsT=wqk_sb[:, ko, rc * P:(rc + 1) * P],
                        rhs=xts[st][:, ko, :],
                        start=(ko == 0),
                        stop=(ko == KO - 1),
                    )
                if koh == 1:
                    nc.vector.tensor_copy(
                        qkT[:, rc, st * PSTRIPE:(st + 1) * PSTRIPE], ps[:]
                    )

            return [(None, lambda: half(0)),
                    (("qk", st, rc), lambda: half(1))]

        def v_sub(st, sub, alt=False):
            """V rows for s-chunk 4*st+sub: two ~0.85us closures."""
            cell = {}

            def half(koh):
                if koh == 0:
                    pool, tg = (ps_y, "ps_y") if alt else (ps_wk, "ps_wk")
                    cell[0] = pool.tile(
                        [P, NH * HD], F32, tag=tg, name=f"pv{st}_{sub}"
                    )
                ps = cell[0]
                for ko in range(4 * koh, 4 * koh + 4):
                    nc.tensor.matmul(
                        ps[:],
                        lhsT=xts[st][:, ko, sub * P:(sub + 1) * P],
                        rhs=wv_sb[:, ko, :],
                        start=(ko == 0),
                        stop=(ko == KO - 1),
                    )
                if koh == 1:
                    sc = st * (PSTRIPE // P) + sub
                    nc.vector.tensor_copy(
                        v_sb[:, sc, :, 0:HD],
                        ps.rearrange("p (h e) -> p h e", h=NH),
                    )

            return [(None, lambda: half(0)),
                    (("v", st * 4 + sub), lambda: half(1))]

        def stripe_fillers(st):
            """All of stripe st's proj pieces in the demand order of
            superblock st's attention: q/k rows for hp0 first, then the v
            chunks (needed from iteration 4*hp0+...), then later head-pairs'
            rows interleaved q,k."""
            out = []
            out += qk_rc(st, 0) + qk_rc(st, 4)
            for sub in range(4):
                out += v_sub(st, sub)
            for hp in range(1, 4):
                out += qk_rc(st, hp) + qk_rc(st, 4 + hp)
            return out

        ySbs = [None] * N_SB

        def out_unit(sb, sub):
            """Output projection for s-rows sb*512+sub*128..+128. Two closures."""
            cell = {}

            def half(nt):
                ps = ps_wk.tile([P, 512], F32, tag="ps_wk", name=f"ops{sb}_{sub}_{nt}")
                ySb = ySbs[sb]
                for cc in range(4):
                    nc.tensor.matmul(
                        ps[:],
                        lhsT=ySb[:, cc, sub * P:(sub + 1) * P],
                        rhs=wout_sb[:, cc, nt * 512:(nt + 1) * 512],
                        start=(cc == 0),
                        stop=(cc == 3),
                    )
                if nt == 0:
                    cell["o_t"] = o_pool.tile([P, 2, 512], F32, tag="osb", name=f"ot{sb}_{sub}")
                o_t = cell["o_t"]
                nc.vector.tensor_copy(o_t[:, nt, :], ps[:])
                if nt == 1:
                    row = (sb * (QSB // P) + sub) * P
                    nc.sync.dma_start(
                        out[row:row + P, :], o_t.rearrange("p a b -> p (a b)")
                    )

            return [(None, lambda: half(0)), (None, lambda: half(1))]

        # global filler deque of (tag, closure); tags mark closures that
        # publish data (qkT rows, v chunks) so consumers can demand-pop
        # them before issuing reads -- correctness no longer depends on
        # pop pacing.
        fillers = []
        done_tags = set()

        def _run_filler():
            tag, fn = fillers.pop(0)
            fn()
            if tag is not None:
                done_tags.add(tag)
            return tag

        def pop_fillers(n):
            for _ in range(n):
                if fillers:
                    _run_filler()

        def flush_fillers():
            while fillers:
                _run_filler()

        def ensure(tag):
            if tag in done_tags:
                return
            while fillers:
                if _run_filler() == tag:
                    return

        pending = []   # deferred normalize multiplies (previous head-pair)

        def attn_sb(sb, tail_reserve=0, pre_norm_hook=None, bg_period=2):
            ySb = y_pool.tile([P, 4, QSB], BF16, tag="ysb", name=f"ysb{sb}")
            ySbs[sb] = ySb
            nch = 4 * (sb + 1)
            ys = {}

            # One-chunk software pipeline over a flattened (head-pair, chunk)
            # stream: QK(next) is issued on the PE queue BEFORE AV(cur), so
            # the next chunk's scores stream while this chunk's exp runs on
            # ACT -- including across head-pair boundaries. A chunk's score
            # tile [P, head, QSB] frees as soon as its own (single, two-head)
            # exp retires, so 2 score bufs suffice for the lookahead.
            def issue_chunk(hp, c):
                heads = (2 * hp, 2 * hp + 1)
                rc_k = 4 + hp
                qo = P * max(0, c - 4 * sb)
                # demand-pop the fillers that publish this chunk's inputs
                ensure(("qk", sb, hp))        # q rows, stripe sb
                ensure(("qk", c // 4, rc_k))  # k rows, chunk stripe
                ensure(("v", c))              # v rows (read 1 iter later)
                s2c = ps_sc.tile([P, 2, QSB], F32, tag="ps_sc",
                                 name=f"s2c{c % 2}")
                for j, h in enumerate(heads):
                    bp = (h % 2) * HD
                    nc.tensor.matmul(
                        s2c[:, j, qo:],
                        lhsT=qkT[bp:bp + HD, rc_k, c * P:(c + 1) * P],
                        rhs=qkT[bp:bp + HD, hp, sb * QSB + qo:(sb + 1) * QSB],
                        start=True,
                        stop=True,
                    )
                pt = pt_pool.tile([P, 2, QSB], BF16, tag="pt")
                nc.scalar.activation(
                    pt[:, :, qo:], s2c[:, :, qo:],
                    mybir.ActivationFunctionType.Exp,
                    scale=SCALE,
                )
                if c >= 4 * sb:
                    for j in range(2):
                        # triangle at the causal diagonal block (DVE)
                        nc.vector.tensor_tensor(
                            pt[:, j, qo:qo + P],
                            pt[:, j, qo:qo + P],
                            tri[:],
                            mybir.AluOpType.mult,
                        )
                return (hp, c, qo, pt)

            def do_av(state):
                hp, c, qo, pt = state
                heads = (2 * hp, 2 * hp + 1)
                if c == 0:
                    # allocated here (not at issue time) so the previous
                    # head-pair's normalize reads are issued before the bank
                    # is recycled
                    ys[hp] = [
                        ps_y.tile([P, QSB], F32, tag="ps_y",
                                  name=f"yps{hp}_{i}")
                        for i in range(2)
                    ]
                for j, (h, y_ps) in enumerate(zip(heads, ys[hp])):
                    nc.tensor.matmul(
                        y_ps[0:HD + 1, qo:],
                        lhsT=v_sb[:, c, h, :],
                        rhs=pt[:, j, qo:],
                        start=(c == 0),
                        stop=(c == nch - 1),
                    )

            def normalize(hp, tail):
                # normalize: DVE copies only -- ACT copies would delay the
                # latency-critical exp chain on the ACT queue; the
                # reciprocal chain runs off base-partition-0 staging
                # (reciprocal_approx_fast reads garbage from nonzero bases).
                heads = (2 * hp, 2 * hp + 1)
                if tail:
                    # shortest DVE chain first (den -> recip -> bf16 cast);
                    # yb copies only feed the final mults, so they queue
                    # after. The sb3 out-proj prestart + reserved fillers
                    # keep the PE (and its p-state) hot throughout; the
                    # 1/den broadcast is a bf16 PE outer product.
                    dens = []
                    for j, y_ps in enumerate(ys[hp]):
                        den = r_pool.tile([1, QSB], F32, tag="den",
                                          name=f"den{j}")
                        nc.vector.tensor_copy(den[:], y_ps[HD:HD + 1, :])
                        dens.append(den)
                    if pre_norm_hook is not None:
                        pre_norm_hook()
                    rbs = []
                    for j in range(2):
                        r = r_pool.tile([1, QSB], F32, tag="r", name=f"r{j}")
                        nc.vector.reciprocal_approx_fast(r[:], dens[j][:])
                        rb = r_pool.tile([1, QSB], BF16, tag="rb",
                                         name=f"rb{j}")
                        nc.vector.tensor_copy(rb[:], r[:])
                        rbs.append(rb)
                    pop_fillers(2)
                    ybs = []
                    for j, y_ps in enumerate(ys[hp]):
                        yb = r_pool.tile([HD, QSB], F32, tag="yb",
                                         name=f"yb{j}")
                        nc.vector.tensor_copy(yb[:], y_ps[0:HD, :])
                        ybs.append(yb)
                    pop_fillers(2)
                    for j, (h, yb) in enumerate(zip(heads, ybs)):
                        bp = (h % 2) * HD
                        rbc_ps = ps_y.tile([P, QSB], F32, tag="ps_y",
                                           name=f"rbcps{j}")
                        nc.tensor.matmul(rbc_ps[0:HD, :], lhsT=ones64[:],
                                         rhs=rbs[j][:], start=True, stop=True)
                        nc.vector.tensor_tensor(
                            ySb[bp:bp + HD, hp, :], yb[:], rbc_ps[0:HD, :],
                            mybir.AluOpType.mult,
                        )
                    return
                ybs = []
                for j, y_ps in enumerate(ys[hp]):
                    den = r_pool.tile([1, QSB], F32, tag="den", name=f"den{j}")
                    nc.vector.tensor_copy(den[:], y_ps[HD:HD + 1, :])
                    yb = r_pool.tile([HD, QSB], F32, tag="yb", name=f"yb{j}")
                    nc.vector.tensor_copy(yb[:], y_ps[0:HD, :])
                    r = r_pool.tile([1, QSB], F32, tag="r", name=f"r{j}")
                    nc.vector.reciprocal_approx_fast(r[:], den[:])
                    rbc = r_pool.tile([HD, QSB], F32, tag="rbc", name=f"rbc{j}")
                    nc.gpsimd.partition_broadcast(rbc[:], r[:])
                    ybs.append((yb, rbc))

                def norm_mults(hp=hp, ybs=ybs, heads=heads):
                    for (h, (yb, rbc)) in zip(heads, ybs):
                        bp = (h % 2) * HD
                        nc.vector.tensor_tensor(
                            ySb[bp:bp + HD, hp, :], yb[:], rbc[:],
                            mybir.AluOpType.mult,
                        )
                pending.append(norm_mults)

            def tags_for(hp_, c_):
                return (("qk", sb, hp_), ("qk", c_ // 4, 4 + hp_),
                        ("v", c_))

            stream = [(hp, c) for hp in range(NH // 2) for c in range(nch)]
            state = issue_chunk(*stream[0])
            for idx, (hp, c) in enumerate(stream):
                nxt = (issue_chunk(*stream[idx + 1])
                       if idx + 1 < len(stream) else None)
                # pacing: pre-pop toward tags needed 4 iterations out
                # (capped, so ensure() never has to burst), else a per-sb
                # background drain -- early superblocks have cheap exps and
                # no ACT slack for extra PE work, late ones have plenty.
                la = stream[min(idx + 4, len(stream) - 1)]
                need = [t for t in tags_for(*la) if t not in done_tags]
                pops = 0
                while need and pops < 2 and fillers:
                    _run_filler()
                    pops += 1
                    need = [t for t in need if t not in done_tags]
                if (pops == 0 and idx % 4 != 3
                        and len(fillers) > tail_reserve):
                    pop_fillers(1)
                if c == 2 and pending:
                    pending.pop(0)()
                do_av(state)
                if c == nch - 1:
                    normalize(hp, tail_reserve and hp == NH // 2 - 1)
                state = nxt

        # ---- phase schedule ----
        # P0 (inline): only what sb0/hp0 needs up front -- q rc0, k rc4,
        # v chunks 0-1. Everything else is demand-popped via `ensure`.
        for tag, fn in (qk_rc(0, 0, alt=True) + qk_rc(0, 4, alt=True)
                        + v_sub(0, 0, alt=True) + v_sub(0, 1)):
            fn()
            if tag is not None:
                done_tags.add(tag)
        # rest of stripe 0 in sb0's demand order, then stripe 1.
        fillers += v_sub(0, 2) + v_sub(0, 3)
        for hp in range(1, 4):
            fillers += qk_rc(0, hp) + qk_rc(0, 4 + hp)

        # sb0 ||| stripe1. No inter-sb filler flushes: the deque carries
        # over so the ACT pipeline never starves while the PE bursts.
        load_stripe(1)
        fillers += stripe_fillers(1)
        attn_sb(0, bg_period=0)

        # sb1 ||| stripe2 + q3 (pulled early) + out0
        load_stripe(2)
        load_stripe(3)
        fillers += stripe_fillers(2)
        for rc in range(4):               # q rows of stripe 3
            fillers += qk_rc(3, rc)
        for sub in range(4):
            fillers += out_unit(0, sub)
        attn_sb(1, bg_period=4)

        # sb2 ||| k3 + out1
        for rc in range(4, 8):            # k rows of stripe 3
            fillers += qk_rc(3, rc)
        for sub in range(4):
            fillers += out_unit(1, sub)
        attn_sb(2, bg_period=2)

        # sb3 ||| v3 + out2; a few closures held back to keep the PE warm
        # through the last normalize so the final out-proj runs at max clock
        for sub in range(4):
            fillers += v_sub(3, sub)
        for sub in range(4):
            fillers += out_unit(2, sub)

        # sb3 out-proj row 0: contraction chunks 0-2 pre-start inside the
        # tail normalize (they only read head-pairs 0-2); the cc3 finisher
        # and rows 1-3 run after. The pre-started halves live in ps_sc --
        # free once the last exp retires, and never touched by fillers.
        o3_cells = {}

        def out3_prestart():
            ySb = ySbs[3]
            for nt in range(2):
                ps = ps_sc.tile([P, 512], F32, tag="ps_sc",
                                name=f"o3s{nt}")
                o3_cells[nt] = ps
                for cc in range(3):
                    nc.tensor.matmul(
                        ps[:],
                        lhsT=ySb[:, cc, 0:P],
                        rhs=wout_sb[:, cc, nt * 512:(nt + 1) * 512],
                        start=(cc == 0),
                        stop=False,
                        skip_group_check=True,
                    )

        attn_sb(3, tail_reserve=6, pre_norm_hook=out3_prestart, bg_period=1)
        flush_fillers()
        for fn in pending:
            fn()
        pending.clear()

        # tail: finish sb3 out-proj row 0, then rows 1-3
        ySb3 = ySbs[3]
        o3_t = o_pool.tile([P, 2, 512], F32, tag="osb", name="o3t")
        for nt in range(2):
            nc.tensor.matmul(
                o3_cells[nt][:],
                lhsT=ySb3[:, 3, 0:P],
                rhs=wout_sb[:, 3, nt * 512:(nt + 1) * 512],
                start=False,
                stop=True,
                skip_group_check=True,
            )
            nc.vector.tensor_copy(o3_t[:, nt, :], o3_cells[nt][:])
        nc.sync.dma_start(
            out[3 * QSB:3 * QSB + P, :], o3_t.rearrange("p a b -> p (a b)")
        )
        for sub in range(1, 4):
            for _tag, fn in out_unit(3, sub):
                fn()


_NC_CACHE = None


def _build_program():
    global _NC_CACHE
    if _NC_CACHE is not None:
        return _NC_CACHE
    nc = bacc.Bacc("TRN2", target_bir_lowering=False, debug=False)
    xT = nc.dram_tensor("xT", [D, S], BF16, kind="ExternalInput").ap()
    w_qk = nc.dram_tensor("w_qk", [D, 1024], BF16, kind="ExternalInput").ap()
    w_v = nc.dram_tensor("w_v", [D, 512], BF16, kind="ExternalInput").ap()
    w_out = nc.dram_tensor("w_out", [512, D], BF16, kind="ExternalInput").ap()
    out = nc.dram_tensor("out", [S, D], F32, kind="ExternalOutput").ap()
    with tile.TileContext(nc) as tc:
        _attention_kernel(tc, out, xT, w_qk, w_v, w_out)
    nc.compile()
    _NC_CACHE = nc
    return nc


def make_in_maps(x, W_qkv, W_out):
    import ml_dtypes

    bf16 = ml_dtypes.bfloat16
    x = np.ascontiguousarray(np.asarray(x, dtype=np.float32))
    W_qkv = np.asarray(W_qkv, dtype=np.float32)
    W_out = np.asarray(W_out, dtype=np.float32)
    in_maps = []
    for c in range(8):
        b, g = divmod(c, 2)
        lo = 512 * g
        cols = np.arange(lo, lo + 512)
        in_maps.append({
            "xT": np.ascontiguousarray(x[b].T).astype(bf16),
            "w_qk": np.ascontiguousarray(
                np.concatenate([W_qkv[:, cols], W_qkv[:, D + cols]], axis=1)
            ).astype(bf16),
            "w_v": np.ascontiguousarray(W_qkv[:, 2 * D + cols]).astype(bf16),
            "w_out": np.ascontiguousarray(W_out[cols, :]).astype(bf16),
        })
    return in_maps


def combine_outputs(results):
    # results: list of 8 dicts with "out" [S, D]; core c = 2*b + g
    return np.stack(
        [results[2 * b]["out"] + results[2 * b + 1]["out"] for b in range(B)]
    ).astype(np.float32)


def kernel(x, W_qkv, W_out):
    nc = _build_program()
    in_maps = make_in_maps(x, W_qkv, W_out)
    res = run_bass_kernel_spmd(nc, in_maps, core_ids=list(range(8)))
    return combine_outputs(res.results)


if __name__ == "__main__":
    # smoke test against a local numpy reference
    rng = np.random.default_rng(0)
    x = rng.standard_normal((B, S, D), dtype=np.float32)
    W_qkv = (rng.standard_normal((D, 3 * D)) * 0.02).astype(np.float32)
    W_out = (rng.standard_normal((D, D)) * 0.02).astype(np.float32)
    out = kernel(x, W_qkv, W_out)
    print("out", out.shape, out.dtype, float(np.abs(out).mean()))

